# revision 135
# baseline (speedup 1.0000x reference)
"""Trainium2 Bass kernel for nn_DyIntraModalityUpdate — fp8 DoubleRow,
software-pipelined, DMA-issue-aware. HW-validated lineage: f32r 537,877 ->
fp8 DR 386,539 (prior session) -> 332,750 -> 328,126 -> 303,692 ->
this layout 298,712 ns, rel err 6.14e-3 vs the 2e-2 gate.

Beyond the prior fp8-DR design, this version:
- consolidates DMA traffic (all masks in 4 upfront DMAs; paired x loads
  [128,2,D]; kq/v column-thirds of w_Xlin staged separately so each
  branch's v-columns stream outside its critical window; paired out DMAs)
  because every DMA holds its issuing sequencer for preamble+transfer --
  the SP queue, not any compute engine, bounded the old schedule;
- pairs psum tenants (transposes, kq projections 2-per-bank with byte-offset
  matmul dsts over base-0 fp8 operands -- legal where base-64 operands are
  not) and retags psum into pss/psu/psS/pso x2 banks each;
- drops the resident bf16 q copy (q reloads per-stage like v, f32r
  transposes) freeing SBUF for double-buffered w_out bf16 tiles, which
  removes the v->q transition stall;
- emits the v4q gate + branch-q weight streams at B(v,2)-start so they
  overlap v-stage compute, with the gate's per-kt weight DMA + matmuls
  interleaved INTO B(v,2)'s g-loop: engine queues are in-order, so a block
  of matmuls each waiting on its own weight tile head-of-line blocks every
  later matmul of the stage (-24us when fixed). Check any emitted-early
  stream for this pattern. (Interleaving the OUT-PROJECTION the same way
  was tried and regresses: its chunk matmuls then wait on per-mp residuals
  inside the loop, a worse block than the 7us end-of-stage batch.)

Hardware rules honored (walrus rejects or device-traps otherwise): no mixed
32/non-32-bit matmul operands; Pool never touches PSUM and runs no
scalar-ptr ops; max one non-scalar PSUM input per instruction; never combine
base-partition-64 operands with a PSUM byte-offset matmul destination; a
[*,512] Exp over a psum bank co-tenanted by a base-64-operand matmul (the
h1|h0 pairing trick) DEVICE-HANGS despite satisfying the documented offset
rule. The refined co-tenancy rule (HW-validated both ways): two matmuls
may share a psum bank iff BOTH have base-0 operands; any base-64-operand
matmul needs its own bank-aligned tile. The two base-0 h0 score quadrants
of each mp pair therefore share a bank with ONE fused [*,512] exp (the
h1 quadrants stay solo); the 2KB pT slot growth this needs is paid by
packing both mask rows into one free-size slot (q's row at partition 64,
with a matching-base ones row for its replication matmul -- matmul
operands must share base partition, one of {0,32,64}). Matmul DSTS may
also be base-64 (bass+CoreSim verified; enables e.g. packing two tiles'
row-sums into one bank for a shared reciprocal -- but deferring the even
mp's normalize to the odd mp's slot serializes the pipeline +20us, so the
per-mp reciprocal stays). ACT Identity with
scale+bias APs and a BF16 dst is safe for the k-eviction (f32r dst traps).
DMA queues: SP plus the ACT HWDGE queue (hw-validated); gpsimd SWDGE-queue
DMAs were never hw-validated here and are avoided (worth only ~1us in sim).
"""


import os
import sys

import numpy as np

for _p in ("/opt/trn_rl_repo", "/root/.axon_site/_ro/trn_rl_repo"):
    if os.path.isdir(_p) and _p not in sys.path:
        sys.path.insert(0, _p)

import concourse.bass as bass  # noqa: E402,F401
import concourse.mybir as mybir  # noqa: E402
import concourse.tile as tile  # noqa: E402
from concourse import bacc  # noqa: E402
from concourse.bass_utils import run_bass_kernel_spmd  # noqa: E402
from concourse.masks import make_identity  # noqa: E402

F32 = mybir.dt.float32
F32R = mybir.dt.float32r
BF16 = mybir.dt.bfloat16
F8 = mybir.dt.float8e4
ALU = mybir.AluOpType
ACTF = mybir.ActivationFunctionType
DR = mybir.MatmulPerfMode.DoubleRow

B_CORE = 4
NTOK = 256
D = 1024
DQKV = 3 * D
NCORES = 8
NEGBIAS = -1e9 / 8.0  # masked_fill(-1e9) then /sqrt(64)
WS = 16.0             # fp8 weight scale
EXP_SCALE = 0.125 / (WS * WS)

WEIGHT_NAMES = ("w_v4q", "b_v4q", "w_q4v", "b_q4v",
                "w_vlin", "b_vlin", "w_qlin", "b_qlin",
                "w_vout", "b_vout", "w_qout", "b_qout")


def build_nc():
    nc = bacc.Bacc("TRN2", target_bir_lowering=False, debug=False)
    dram = {}

    def din(name, shape):
        dram[name] = nc.dram_tensor(name, shape, F32, kind="ExternalInput").ap()

    def dout(name, shape):
        dram[name] = nc.dram_tensor(name, shape, F32, kind="ExternalOutput").ap()

    din("v", [B_CORE, NTOK, D])
    din("q", [B_CORE, NTOK, D])
    din("v_mask", [B_CORE, NTOK])
    din("q_mask", [B_CORE, NTOK])
    for g in ("v4q", "q4v"):
        din(f"w_{g}", [D, D])
        din(f"b_{g}", [D])
    for x in ("v", "q"):
        din(f"w_{x}lin", [D, DQKV])
        din(f"b_{x}lin", [DQKV])
        din(f"w_{x}out", [D, D])
        din(f"b_{x}out", [D])
    dout("out_v", [B_CORE, NTOK, D])
    dout("out_q", [B_CORE, NTOK, D])

    with tile.TileContext(nc) as tc:
        with tc.tile_pool(name="cpool", bufs=1) as cpool, \
             tc.tile_pool(name="wpool", bufs=1) as wpool, \
             tc.tile_pool(name="pspool", bufs=8, space="PSUM") as ps, \
             tc.tile_pool(name="apool", bufs=1) as apool:
            # ---- constants ----
            ones_f = cpool.tile([128, 128], F32, name="ones_f")
            nc.gpsimd.memset(ones_f[:], 1.0)
            ones128 = cpool.tile([128, 128], F32R, name="ones128")
            nc.vector.tensor_copy(ones128[:], ones_f[:])
            ones1 = cpool.tile([1, 128], F32R, name="ones1")
            nc.vector.tensor_copy(ones1[:], ones_f[0:1, :])
            ones65 = cpool.tile([65, 128], F32R, name="ones65")
            nc.vector.tensor_copy(ones65[:], ones_f[0:65, :])
            ident_f = cpool.tile([128, 128], F32, name="ident_f")
            make_identity(nc, ident_f[:])
            ident = cpool.tile([128, 128], F32R, name="ident")
            nc.vector.tensor_copy(ident[:], ident_f[:])
            zero_f = cpool.tile([128, 4], F32, name="zero_f")
            nc.gpsimd.memset(zero_f[:], 0.0)
            bor = {}

            # ---- all masks in 4 upfront DMAs (row-major and token-major) ----
            # one shared free-size slot; q's row sits at partition 64 so
            # its slices stay legal matmul operands (base must be 0/32/64)
            mrow_pack = cpool.tile([65, B_CORE * NTOK], F32R,
                                   name="mrow_pack")
            mrow_all = {}
            mcolT_all = {}
            for xi, X in zip((0, 64), ("v", "q")):
                nc.sync.dma_start(
                    mrow_pack[xi:xi + 1, :],
                    dram[f"{X}_mask"].rearrange("b t -> (b t)")
                    .bitcast(F32R).unsqueeze(0))
                mrow_all[X] = mrow_pack[xi:xi + 1, :]
                ct = cpool.tile([128, B_CORE, 2], F32, name=f"mcolT_{X}")
                nc.sync.dma_start(
                    ct[:], dram[f"{X}_mask"].rearrange("b (j p) -> p b j", p=128))
                mcolT_all[X] = ct

            bw = {}

            def load_wlin_tile(X, kt, part="kq"):
                """Stage the kq-columns (0:2048) or v-columns (2048:3072) of
                one 128-row block of w_Xlin (f32), cast to the fp8 pack at
                16x scale. The v-columns are deferred out of each branch's
                critical window (scores need only kq). Branch v rides the
                prologue-idle ACT queue (even kt, ACT cast) and Pool (odd kt,
                DVE cast); branch q stages over Pool's queue mid-run."""
                wl = bw[X][0]
                i, j = divmod(kt, 2)
                if X == "v":
                    eng = nc.scalar if kt % 2 == 0 else nc.sync
                else:
                    eng = nc.sync
                c0, c1 = (0, 2048) if part == "kq" else (2048, 3072)
                st = wpool.tile([128, c1 - c0], F32,
                                name=f"wst_{X}_{kt}_{part}", tag="wstage",
                                bufs=2)
                eng.dma_start(
                    st[:], dram[f"w_{X}lin"][kt * 128:(kt + 1) * 128, c0:c1])
                dst = wl[i][:, j, c0:c1]
                if kt % 2 == 0:
                    nc.scalar.activation(dst, st[:], ACTF.Copy, scale=WS)
                else:
                    nc.vector.tensor_scalar(dst, st[:], WS, None, ALU.mult)

            def alloc_branch_weights(X):
                wl = [wpool.tile([128, 2, DQKV], F8, name=f"wl_{X}_{i}",
                                 tag=f"wl{X}{i}", bufs=1) for i in range(4)]
                bw[X] = [wl, None, None, None, None]

            def load_branch_smalls(X):
                """Small per-branch tensors: 16x kq bias, replicated 16x v
                bias and out bias (PE K=1 matmuls + ACT evictions)."""
                blin_d = dram[f"b_{X}lin"]
                bout_d = dram[f"b_{X}out"]
                b_kq = wpool.tile([128, 16], F32, name=f"bkq_{X}", tag="bkq",
                                  bufs=1)
                nc.sync.dma_start(b_kq[:],
                                  blin_d[0:2048].rearrange("(o p) -> p o", p=128))
                b16 = wpool.tile([128, 16], F32, name=f"b16_{X}", tag="b16",
                                 bufs=2)
                nc.vector.tensor_scalar_mul(b16[:], b_kq[:], WS)
                b_o = wpool.tile([1, D], F32R, name=f"bo_{X}", tag="bo", bufs=1)
                nc.sync.dma_start(b_o[:], bout_d.bitcast(F32R).unsqueeze(0))
                # v-bias rides the residual: softmax weights sum to 1, so
                # u_true = attn_avg(v) + b_v exactly; store b_v feature-major
                bvT = wpool.tile([128, 8], F32, name=f"bvT_{X}", tag="bvT",
                                 bufs=2)
                nc.sync.dma_start(bvT[:],
                                  blin_d[2048:3072].rearrange("(o p) -> p o", p=128))
                br = wpool.tile([128, D], F32, name=f"bor_{X}", tag="bor", bufs=2)
                for h in range(2):
                    psc = ps.tile([128, 512], F32, name=f"psbo_{X}{h}", tag="pso", bufs=2)
                    nc.tensor.matmul(psc[:], ones1[:],
                                     b_o[:, h * 512:(h + 1) * 512],
                                     start=True, stop=True)
                    nc.scalar.copy(br[:, h * 512:(h + 1) * 512], psc[:])
                bor[X] = br
                bw[X][1:5] = [b16, bvT, None, dram[f"w_{X}out"]]

            meanT = {}
            g2T = {}

            def emit_gate(gname, dst, src_meanT, tag_prefix):
                """gate = sigmoid(relu(mean) @ w + b); store (1+gate)^2
                transposed as g2T[dst] [128, 8, 4] (fp32, per-partition use).
                Weight stream: one [128, 1024] DMA per row block, 4-deep, on
                Pool's queue (cheap issue, keeps SP free)."""
                w_d = dram[f"w_{gname}"]
                b_d = dram[f"b_{gname}"]
                bg = apool.tile([1, D], F32R, name=f"bg_{gname}",
                                tag="gtmp", bufs=2)
                nc.sync.dma_start(bg[:], b_d.bitcast(F32R).unsqueeze(0))
                gsb = apool.tile([4, D], F32, name=f"g_{gname}",
                                 tag="gtmp", bufs=2)
                psg = [ps.tile([4, 512], F32, name=f"psg_{gname}{h}", tag="pso", bufs=2)
                       for h in range(2)]
                for kt in range(8):
                    wgt = apool.tile([128, D], F32R,
                                     name=f"wg_{gname}_{kt}", tag="wg",
                                     bufs=2)
                    nc.sync.dma_start(
                        wgt[:], w_d[kt * 128:(kt + 1) * 128, :].bitcast(F32R))
                    for h in range(2):
                        nc.tensor.matmul(psg[h][:], src_meanT[:, kt, :],
                                         wgt[:, h * 512:(h + 1) * 512],
                                         start=(kt == 0), stop=False)
                for h in range(2):
                    nc.tensor.matmul(psg[h][:], ones1[0:1, 0:4],
                                     bg[:, h * 512:(h + 1) * 512],
                                     start=False, stop=True)
                    nc.scalar.activation(gsb[:, h * 512:(h + 1) * 512], psg[h][:],
                                         ACTF.Sigmoid)
                nc.vector.tensor_scalar_add(gsb[:], gsb[:], 1.0)
                g2 = apool.tile([4, D], F32R, name=f"g2_{gname}",
                                tag="g2", bufs=2)
                nc.vector.tensor_tensor(g2[:], gsb[:], gsb[:], ALU.mult)
                gt = wpool.tile([128, 8, 4], F32, name=f"g2T_{dst}")
                for c in range(8):
                    pst = ps.tile([128, 4], F32R, name=f"psgt_{gname}{c}", tag="psS", bufs=2)
                    nc.tensor.transpose(pst[:], g2[:, c * 128:(c + 1) * 128],
                                        ident[0:4, 0:4])
                    nc.vector.tensor_copy(gt[:, c, :], pst[:])
                g2T[dst] = gt

            # ---- main: software-pipelined over 8 (branch, batch) stages;
            # stage A (loads/transposes/projections) of stage i+1 interleaves
            # with stage B (attention/out-projection) of stage i ----
            qres = {}
            fvg = {}
            vtok = {}
            wob = {}
            bgp = {}
            accv = apool.tile([4, D], F32, name="accv", tag="accv", bufs=1)
            accn = apool.tile([4, 2], F32, name="accn", tag="accn", bufs=1)

            def make_bgp(X):
                """Per-partition bias*gate for the ACT Identity k-eviction."""
                gt = g2T[X]
                b16 = bw[X][1]
                t = wpool.tile([128, 8, 4], F32, name=f"bgp_{X}", tag="bgp",
                               bufs=2)
                for c in range(8):
                    nc.vector.tensor_scalar_mul(t[:, c, :], gt[:, c, :],
                                                b16[:, c:c + 1])
                bgp[X] = t

            def load_wout(X):
                """w_Xout resident as bf16 (cast on Pool); ends the per-batch
                4MB re-stream of the f32r out-projection weights."""
                wout_d = dram[f"w_{X}out"]
                eng = nc.sync
                tiles = []
                for kt in range(8):
                    st = wpool.tile([128, D], F32, name=f"wos_{X}_{kt}",
                                    tag="wstage", bufs=2)
                    eng.dma_start(st[:], wout_d[kt * 128:(kt + 1) * 128, :])
                    t = wpool.tile([128, D], BF16, name=f"wob_{X}_{kt}",
                                   tag=f"wob{kt}", bufs=2)
                    nc.gpsimd.tensor_scalar_add(t[:], st[:], 0.0)
                    tiles.append(t)
                wob[X] = tiles

            def stage_A_loads(X, b):
                """x loads (v only; q reads the resident bf16 tiles), mask
                prep from the upfront consolidated mask tiles, v-mean
                accumulation; allocates the stage's tiles."""
                st = {"X": X, "b": b}
                if X == "q" and b == 0:
                    for i in range(2):
                        vt = apool.tile([128, 2, 16, 128], F8,
                                        name=f"vtok_q_{i}", tag=f"vtok{i}",
                                        bufs=1)
                        nc.gpsimd.memset(vt[:, :, :, 64:128], WS)
                        vtok[("q", i)] = vt
                x2 = apool.tile([128, 2, D], F32R, name=f"x_{X}_{b}",
                                tag="xt", bufs=2)
                nc.sync.dma_start(
                    x2[:], dram[X][b].bitcast(F32R)
                    .rearrange("(j p) d -> p j d", p=128))
                xt = [x2[:, 0, :], x2[:, 1, :]]
                st["xt"] = xt
                mrow = mrow_all[X][:, b * NTOK:(b + 1) * NTOK]
                obase = 0 if X == "v" else 64
                psmr = ps.tile([128, NTOK], F32, name=f"psmr_{X}_{b}", tag="psS", bufs=2)
                nc.tensor.matmul(psmr[:], ones65[obase:obase + 1, :], mrow,
                                 start=True, stop=True)
                maskrep = apool.tile([128, NTOK], BF16, name=f"maskrep_{X}_{b}",
                                     tag="maskrep", bufs=1)
                nc.scalar.copy(maskrep[:], psmr[:])
                st["maskrep"] = maskrep
                mb = []
                for jt in range(2):
                    t = apool.tile([128, 1], F32, name=f"mbias_{X}_{b}_{jt}",
                                   tag="mbias", bufs=4)
                    nc.vector.tensor_scalar(t[:], mcolT_all[X][:, b, jt:jt + 1],
                                            1.0, -NEGBIAS,
                                            ALU.subtract, ALU.mult)
                    mb.append(t)
                st["mb"] = mb

                if X == "v":
                    # accumulate v masked-sums from this batch's x tiles
                    pm = [ps.tile([4, 512], F32, name=f"pmv_{b}{h}", tag="pso", bufs=2)
                          for h in range(2)]
                    pn = ps.tile([4, 2], F32, name=f"pnv_{b}", tag="psS", bufs=2)
                    for jt in range(2):
                        mc = apool.tile([128, 4], F32R, name=f"mcv_{b}_{jt}",
                                        tag="mcv", bufs=2)
                        nc.vector.tensor_copy(mc[:], zero_f[:])
                        nc.vector.tensor_copy(mc[:, b:b + 1],
                                              mcolT_all[X][:, b, jt:jt + 1])
                        for h in range(2):
                            nc.tensor.matmul(pm[h][:], mc[:],
                                             xt[jt][:, h * 512:(h + 1) * 512],
                                             start=(jt == 0), stop=(jt == 1))
                        nc.tensor.matmul(pn[:], mc[:], ones128[:, 0:2],
                                         start=(jt == 0), stop=(jt == 1))
                    if b == 0:
                        for h in range(2):
                            nc.vector.tensor_copy(
                                accv[:, h * 512:(h + 1) * 512], pm[h][:])
                        nc.vector.tensor_copy(accn[:], pn[:])
                    else:
                        for h in range(2):
                            nc.vector.tensor_tensor(
                                accv[:, h * 512:(h + 1) * 512],
                                accv[:, h * 512:(h + 1) * 512], pm[h][:],
                                ALU.add)
                        nc.vector.tensor_tensor(accn[:], accn[:], pn[:], ALU.add)

                st["vt"] = vtok[(X, b % 2)]
                st["xTraw"] = apool.tile([128, 8, NTOK], BF16,
                                         name=f"xTraw_{X}_{b}", tag="xTraw",
                                         bufs=2)
                st["xTrelu"] = apool.tile([128, 8, NTOK], F8,
                                          name=f"xTrelu_{X}_{b}", tag="xTrelu",
                                          bufs=2)
                st["k_ts"] = {}
                st["q_ts"] = {}
                return st

            def stage_A_chunk(st, i):
                """Chunk i of stage A compute: 0-3 transpose pairs, 4-5 the
                k,q projections (consumed by the NEXT B stage), 6-7
                v-projection halves (consumed one stage later)."""
                X, b = st["X"], st["b"]
                xt = st["xt"]
                if i < 4:
                    # two feature chunks share one psum bank; ONE relu covers
                    # both ([*,512] on ACT beats 2x [*,256] by the access init)
                    pst = ps.tile([128, 2, NTOK], F32R,
                                  name=f"pstp_{X}_{b}_{i}", tag="psS", bufs=2)
                    idm = ident
                    for ci, c in enumerate((2 * i, 2 * i + 1)):
                        for jt in range(2):
                            nc.tensor.transpose(
                                pst[:, ci, jt * 128:(jt + 1) * 128],
                                xt[jt][:, c * 128:(c + 1) * 128], idm[:])
                    nc.scalar.activation(st["xTrelu"][:, 2 * i:2 * i + 2, :],
                                         pst[:, :, :], ACTF.Relu)
                    for ci, c in enumerate((2 * i, 2 * i + 1)):
                        nc.vector.tensor_scalar(st["xTraw"][:, c, :],
                                                pst[:, ci, :],
                                                bw[X][2][:, c:c + 1], None,
                                                ALU.add)
                elif i >= 6:
                    jt = i - 6
                    wl = bw[X][0]
                    vt = st["vt"]
                    for ch in range(2):
                        psv = ps.tile([128, 512], F32,
                                      name=f"psv_{X}_{b}_{jt}_{ch}", tag="psS", bufs=2)
                        for k in range(4):
                            nc.tensor.matmul(
                                psv[:],
                                st["xTrelu"][:, 2 * k:2 * k + 2,
                                             jt * 128:(jt + 1) * 128],
                                wl[k][:, :, 2048 + ch * 512:2048 + (ch + 1) * 512],
                                start=(k == 0), stop=(k == 3), perf_mode=DR)
                        dst = vt[:, jt, ch * 8:(ch + 1) * 8, 0:64]
                        if ch == 0:
                            nc.scalar.copy(dst, psv[:])
                        else:
                            nc.vector.tensor_copy(dst, psv[:])
                else:
                    wl = bw[X][0]
                    b16 = bw[X][1]
                    gate = g2T[X]
                    for pp in range(4):
                        p0 = 4 * (i - 4) + pp
                        # part pair shares one psum bank (base-0 fp8 operands
                        # keep the byte-offset dst legal)
                        psq = ps.tile([128, 2, NTOK], F32,
                                      name=f"pskq_{X}_{b}_{p0}", tag="psS",
                                      bufs=2)
                        for pi, part in enumerate((p0, p0 + 8)):
                            for k in range(4):
                                nc.tensor.matmul(
                                    psq[:, pi, :],
                                    wl[k][:, :, part * 128:(part + 1) * 128],
                                    st["xTrelu"][:, 2 * k:2 * k + 2, :],
                                    start=(k == 0), stop=(k == 3), perf_mode=DR)
                        for pi, part in enumerate((p0, p0 + 8)):
                            if part < 8:
                                t = apool.tile([128, NTOK], BF16,
                                               name=f"k_{X}_{b}_{part}",
                                               tag=f"k{part}", bufs=1)
                                # (psum+bias)*(1+gate)^2 = psum*g2 + bgp on ACT
                                nc.scalar.activation(
                                    t[:], psq[:, pi, :], ACTF.Identity,
                                    bias=bgp[X][:, part, b:b + 1],
                                    scale=gate[:, part, b:b + 1])
                                st["k_ts"][part] = t
                            else:
                                t = apool.tile([128, NTOK], BF16,
                                               name=f"q_{X}_{b}_{part}",
                                               tag=f"q{part - 8}", bufs=1)
                                # (16x psum + 16x bias) * token_mask
                                nc.vector.scalar_tensor_tensor(
                                    t[:], psq[:, pi, :], b16[:, part:part + 1],
                                    st["maskrep"][:], ALU.add, ALU.mult)
                                st["q_ts"][part - 8] = t

            def stage_B_scores_pair(st, g):
                """Scores for one mp pair (2g, 2g+1). Per key chunk jt: the
                two h1 matmuls (base-64 operands) each get an own bank-aligned
                psum tile (base-64 co-tenancy device-hangs); the two h0
                matmuls (base-0 operands, the hw-proven co-tenancy class)
                share one bank and ONE [*,512] exp covers both -- same jt
                means the same per-key mask bias. pT slots per jt:
                0=mp0 h1, 1=mp1 h1, 2=mp0 h0, 3=mp1 h0."""
                X, b = st["X"], st["b"]
                mp0, mp1 = 2 * g, 2 * g + 1
                mb = st["mb"]
                pT = apool.tile([128, 2, 4, NTOK], F8,
                                name=f"pT_{X}_{b}_{g}", tag="pT", bufs=2)
                st[f"pTg{g}"] = pT
                for jt in range(2):
                    for pi, mp in enumerate((mp0, mp1)):
                        k_t, q_t = st["k_ts"][mp], st["q_ts"][mp]
                        pss = ps.tile([128, NTOK], F32,
                                      name=f"pss1_{X}_{b}_{mp}_{jt}",
                                      tag="pss", bufs=2)
                        nc.tensor.matmul(
                            pss[:], k_t[64:128, jt * 128:(jt + 1) * 128],
                            q_t[64:128, :], start=True, stop=True)
                        nc.scalar.activation(
                            pT[:, jt, pi, :], pss[:], ACTF.Exp,
                            bias=mb[jt][:], scale=EXP_SCALE)
                    pss0 = ps.tile([128, 2, NTOK], F32,
                                   name=f"pss0_{X}_{b}_{g}_{jt}",
                                   tag="pss", bufs=2)
                    for pi, mp in enumerate((mp0, mp1)):
                        k_t, q_t = st["k_ts"][mp], st["q_ts"][mp]
                        nc.tensor.matmul(
                            pss0[:, pi, :],
                            k_t[0:64, jt * 128:(jt + 1) * 128],
                            q_t[0:64, :], start=True, stop=True)
                    nc.scalar.activation(
                        pT[:, jt, 2:4, :], pss0[:, :, :], ACTF.Exp,
                        bias=mb[jt][:], scale=EXP_SCALE)

            def stage_B_update(st, mp):
                """One DoubleRow matmul per head (update rows 0:64, replicated
                16x row-sums 64:128), DVE-divide normalize, residual on Pool."""
                X, b = st["X"], st["b"]
                vt = st["vt"]
                g, pi = divmod(mp, 2)
                pT = st[f"pTg{g}"]
                u_tmp = apool.tile([128, NTOK], BF16, name=f"ut_{X}_{b}_{mp}",
                                   tag="utmp", bufs=2)
                psu = ps.tile([128, 512], F32, name=f"psu_{X}_{b}_{mp}",
                              tag="psu", bufs=2)
                for h_loc in range(2):
                    h = 2 * mp + h_loc
                    c0 = h_loc * 256
                    slot = pi if h_loc == 1 else 2 + pi
                    nc.tensor.matmul(psu[:, c0:c0 + 256], vt[:, :, h, :],
                                     pT[:, :, slot, :],
                                     start=True, stop=True, perf_mode=DR)
                # hw allows one PSUM input per op: reciprocal of the
                # replicated row-sums (rows 64:128) then two multiplies
                rinv = apool.tile([64, 512], F32, name=f"ri_{X}_{b}_{mp}",
                                  tag="rinv", bufs=2)
                nc.vector.reciprocal(rinv[:], psu[64:128, :])
                for h_loc in range(2):
                    c0 = h_loc * 256
                    r0 = h_loc * 64
                    nc.vector.tensor_tensor(
                        u_tmp[r0:r0 + 64, :], psu[0:64, c0:c0 + 256],
                        rinv[:, c0:c0 + 256], ALU.mult)
                nc.gpsimd.tensor_tensor(st["xTraw"][:, mp, :],
                                        st["xTraw"][:, mp, :], u_tmp[:], ALU.add)

            def stage_B_outproj_ch(st, ch):
                """Out-projection column half: 2 psum accumulators over all
                8 feature chunks, bias-add eviction into the stage's paired
                out tile; ch==1 fires the two row-half DMAs."""
                X, b = st["X"], st["b"]
                out_d = dram[f"out_{X}"]
                if ch == 0:
                    st["osb"] = [apool.tile([128, D], F32,
                                            name=f"osb_{X}_{b}_{it}",
                                            tag="osb", bufs=2)
                                 for it in range(2)]
                osb = st["osb"]
                pso = [ps.tile([128, 512], F32, name=f"pso_{X}_{b}_{ch}_{it}",
                               tag="pso", bufs=2) for it in range(2)]
                for kt in range(8):
                    for it in range(2):
                        nc.tensor.matmul(pso[it][:],
                                         st["xTraw"][:, kt, it * 128:(it + 1) * 128],
                                         wob[X][kt][:, ch * 512:(ch + 1) * 512],
                                         start=(kt == 0), stop=(kt == 7))
                for it in range(2):
                    nc.vector.tensor_tensor(osb[it][:, ch * 512:(ch + 1) * 512],
                                            pso[it][:],
                                            bor[X][:, ch * 512:(ch + 1) * 512],
                                            ALU.add)
                if ch == 1:
                    for it in range(2):
                        nc.sync.dma_start(
                            out_d[b, it * 128:(it + 1) * 128, :], osb[it][:])

            def finish_v_gate():
                """v-mean finish + v4q gate + branch-q smalls: emitted after
                B(v,2) so the gate weight stream and matmuls overlap B(v,3)."""
                for kt in (6, 7):
                    load_wlin_tile("q", kt)
                for kt in range(8):
                    load_wlin_tile("q", kt, "v")
                load_branch_smalls("q")
                recn = apool.tile([4, 1], F32, name="recn_v", tag="recnv", bufs=1)
                nc.vector.reciprocal(recn[:], accn[:, 0:1])
                rmean = apool.tile([4, D], F32R, name="rmean_v", tag="gtmp", bufs=2)
                nc.vector.tensor_scalar(rmean[:], accv[:], recn[:], 0.0,
                                        ALU.mult, ALU.max)
                mt = wpool.tile([128, 8, 4], F32R, name="meanT_v")
                for c in range(8):
                    pst = ps.tile([128, 4], F32R, name=f"psmt_v{c}", tag="psS", bufs=2)
                    nc.tensor.transpose(pst[:], rmean[:, c * 128:(c + 1) * 128],
                                        ident[0:4, 0:4])
                    nc.vector.tensor_copy(mt[:, c, :], pst[:])
                meanT["v"] = mt
                # v4q gate: bias + psum accumulators now; the per-kt weight
                # DMA + matmuls are interleaved into B(v,2)'s g-loop so the
                # in-order PE queue never head-of-line blocks on the stream
                bg = apool.tile([1, D], F32R, name="bg_v4q", tag="gtmp",
                                bufs=2)
                nc.sync.dma_start(bg[:], dram["b_v4q"].bitcast(F32R).unsqueeze(0))
                fvg["bg"] = bg
                fvg["psg"] = [ps.tile([4, 512], F32, name=f"psg_v4q{h}",
                                      tag="pso", bufs=2) for h in range(2)]

            def v4q_step(kt):
                wgt = apool.tile([128, D], F32R, name=f"wg_v4q_{kt}", tag="wg",
                                 bufs=2)
                nc.sync.dma_start(
                    wgt[:], dram["w_v4q"][kt * 128:(kt + 1) * 128, :]
                    .bitcast(F32R))
                for h in range(2):
                    nc.tensor.matmul(fvg["psg"][h][:], meanT["v"][:, kt, :],
                                     wgt[:, h * 512:(h + 1) * 512],
                                     start=(kt == 0), stop=False)

            def finish_v_gate_post():
                psg, bg = fvg["psg"], fvg["bg"]
                gsb = apool.tile([4, D], F32, name="g_v4q", tag="gtmp", bufs=2)
                for h in range(2):
                    nc.tensor.matmul(psg[h][:], ones1[0:1, 0:4],
                                     bg[:, h * 512:(h + 1) * 512],
                                     start=False, stop=True)
                    nc.scalar.activation(gsb[:, h * 512:(h + 1) * 512],
                                         psg[h][:], ACTF.Sigmoid)
                nc.vector.tensor_scalar_add(gsb[:], gsb[:], 1.0)
                g2 = apool.tile([4, D], F32R, name="g2_v4q", tag="g2", bufs=2)
                nc.vector.tensor_tensor(g2[:], gsb[:], gsb[:], ALU.mult)
                gt = wpool.tile([128, 8, 4], F32, name="g2T_q")
                for c in range(8):
                    pst = ps.tile([128, 4], F32R, name=f"psgt_v4q{c}",
                                  tag="psS", bufs=2)
                    nc.tensor.transpose(pst[:], g2[:, c * 128:(c + 1) * 128],
                                        ident[0:4, 0:4])
                    nc.vector.tensor_copy(gt[:, c, :], pst[:])
                g2T["q"] = gt
                make_bgp("q")
                load_wout("q")

            def emit_prologue_means():
                """q masked-means -> meanT_q; q tiles resident bf16."""
                # q tiles stay resident as bf16 for branch q ----
                x_d = dram["q"]
                ps_mean = [ps.tile([4, 512], F32, name=f"psmean_q{h}", tag="pso", bufs=2)
                           for h in range(2)]
                ps_n = ps.tile([4, 2], F32, name="psn_q", tag="psS", bufs=2)
                for b in range(B_CORE):
                    x2 = apool.tile([128, 2, D], F32R, name=f"mx_q_{b}",
                                    tag="xt", bufs=2)
                    nc.sync.dma_start(
                        x2[:], x_d[b].bitcast(F32R)
                        .rearrange("(j p) d -> p j d", p=128))
                    for jt in range(2):
                        mc = apool.tile([128, 4], F32R, name=f"mc_q_{b}_{jt}",
                                        tag="mc", bufs=2)
                        nc.vector.tensor_copy(mc[:], zero_f[:])
                        nc.vector.tensor_copy(mc[:, b:b + 1],
                                              mcolT_all["q"][:, b, jt:jt + 1])
                        first = (b == 0 and jt == 0)
                        last = (b == B_CORE - 1 and jt == 1)
                        for h in range(2):
                            nc.tensor.matmul(ps_mean[h][:], mc[:],
                                             x2[:, jt, h * 512:(h + 1) * 512],
                                             start=first, stop=last)
                        nc.tensor.matmul(ps_n[:], mc[:], ones128[:, 0:2],
                                         start=first, stop=last)
                recn = apool.tile([4, 1], F32, name="recn_q", tag="recn", bufs=1)
                nc.vector.reciprocal(recn[:], ps_n[:, 0:1])
                rmean = apool.tile([4, D], F32R, name="rmean_q", tag="gtmp", bufs=2)
                for h in range(2):
                    # relu(masked_sum / n): (psum * recn) max 0
                    nc.vector.tensor_scalar(rmean[:, h * 512:(h + 1) * 512],
                                            ps_mean[h][:], recn[:], 0.0,
                                            ALU.mult, ALU.max)
                mt = wpool.tile([128, 8, 4], F32R, name="meanT_q")
                for c in range(8):
                    pst = ps.tile([128, 4], F32R, name=f"psmt_q{c}", tag="psS", bufs=2)
                    nc.tensor.transpose(pst[:], rmean[:, c * 128:(c + 1) * 128],
                                        ident[0:4, 0:4])
                    nc.vector.tensor_copy(mt[:, c, :], pst[:])
                meanT["q"] = mt

                # q4v gate scales branch v; branch-v fp8 weights load+cast during
                # the gate's weight stream

            # ---- emission ----
            alloc_branch_weights("v")
            alloc_branch_weights("q")
            for i in range(2):
                vt = apool.tile([128, 2, 16, 128], F8, name=f"vtok_v_{i}",
                                tag=f"vtok{i}", bufs=1)
                nc.gpsimd.memset(vt[:, :, :, 64:128], WS)
                vtok[("v", i)] = vt
            CUT = "all"
            load_branch_smalls("v")
            sts = {}
            sts[("v", 0)] = stage_A_loads("v", 0)
            for i in range(4):
                stage_A_chunk(sts[("v", 0)], i)   # transposes: gate-free PE work
            emit_prologue_means()
            emit_gate("q4v", "v", meanT["q"], "p")
            if CUT != "gate":
                for kt in range(8):
                    load_wlin_tile("v", kt)
                for kt in range(8):
                    load_wlin_tile("v", kt, "v")
                make_bgp("v")
            if CUT == "all":
                for i in range(4, 8):
                    stage_A_chunk(sts[("v", 0)], i)   # kq (needs gate+w), vproj

            stages = [("v", b) for b in range(B_CORE)] + \
                     [("q", b) for b in range(B_CORE)]
            
            import os as _os
            SEQ = False
            for idx, (X, b) in enumerate(stages):
                if (X, b) not in sts:
                    sts[(X, b)] = stage_A_loads(X, b)
                    for i in range(8):
                        stage_A_chunk(sts[(X, b)], i)
                nxt = stages[idx + 1] if idx + 1 < len(stages) else None
                if SEQ:
                    nxt = None
                if nxt is not None:
                    sts[nxt] = stage_A_loads(*nxt)
                if (X, b) == ("v", 0):
                    load_wout("v")
                if (X, b) == ("v", 2):
                    # v4q gate + branch-q residuals: emitted here so the
                    # weight streams overlap B(v,2)/B(v,3) instead of
                    # stalling the v->q transition
                    finish_v_gate()
                cur = sts.pop((X, b))
                for g in range(4):
                    stage_B_scores_pair(cur, g)
                    if nxt is not None:
                        stage_A_chunk(sts[nxt], 2 * g)
                    if (X, b) == ("v", 2):
                        v4q_step(2 * g)
                    stage_B_update(cur, 2 * g)
                    stage_B_update(cur, 2 * g + 1)
                    if nxt is not None:
                        stage_A_chunk(sts[nxt], 2 * g + 1)
                    if (X, b) == ("v", 2):
                        v4q_step(2 * g + 1)
                if (X, b) == ("v", 2):
                    finish_v_gate_post()
                stage_B_outproj_ch(cur, 0)
                stage_B_outproj_ch(cur, 1)
                # branch-q fp8 weight prefetch + the v4q gate ride branch v
                if X == "v" and b in (0, 1):
                    for kt in (3 * b, 3 * b + 1, 3 * b + 2):
                        load_wlin_tile("q", kt)

    nc.compile()
    return nc


_NC = None


def _get_nc():
    global _NC
    if _NC is None:
        _NC = build_nc()
    return _NC


def run(inputs, trace=False):
    nc = _get_nc()
    in_maps = []
    for c in range(NCORES):
        sl = slice(B_CORE * c, B_CORE * (c + 1))
        m = {"v": np.ascontiguousarray(np.asarray(inputs["v"], dtype=np.float32)[sl]),
             "q": np.ascontiguousarray(np.asarray(inputs["q"], dtype=np.float32)[sl]),
             "v_mask": np.ascontiguousarray(
                 np.asarray(inputs["v_mask"], dtype=np.float32)[sl]),
             "q_mask": np.ascontiguousarray(
                 np.asarray(inputs["q_mask"], dtype=np.float32)[sl])}
        for name in WEIGHT_NAMES:
            m[name] = np.ascontiguousarray(np.asarray(inputs[name], dtype=np.float32))
        in_maps.append(m)
    res = run_bass_kernel_spmd(nc, in_maps, core_ids=list(range(NCORES)),
                               trace=trace)
    uv = np.concatenate([res.results[c]["out_v"] for c in range(NCORES)], axis=0)
    uq = np.concatenate([res.results[c]["out_q"] for c in range(NCORES)], axis=0)
    return (uv, uq), res


def kernel(**inputs):
    (uv, uq), _ = run(inputs, trace=False)
    return uv, uq

